# revision 33
# baseline (speedup 1.0000x reference)
"""Trainium2 Bass kernel for nn_EwaldBlock (gnn_message_passing).

Sharding: data-parallel over graphs. 16 graphs -> 8 cores, 2 whole graphs per
core (paired large+small, padded to static 128-aligned slots), so the
segment_sum stays core-local. Small weights are replicated.

Per-core device program (feature-major activations [E=2x128 part-tiles, n]):
  dot (PE, fp32) -> sin/cos via range-reduced ACT Sin (fused C|S layout)
  z1 = h @ Wp0.T (f32r), silu; z2 atom-major via activation-stationary
  matmuls, silu; hres = 0.6*h + silu(z2)   (GemNet 1/0.6 & 1/sqrt2 scales
  folded into host-preprocessed weights)
  sf (segment outer-product sums): one K=128 matmul per 128-atom tile emits
  sf_r (psum rows 0:64) and sf_i (rows 64:128) at once
  gather-back: one K=128 matmul per chunk = filt_r.T@cosT + filt_i.T@sinT
  e0 dense + 3 residual layers, feature-major f32r matmuls, pipelined as two
  independent n-streams so ACT(silu) and PE overlap across layers.
"""

import os
import sys
import math
import numpy as np

sys.path.insert(0, "/opt/trn_rl_repo")

import concourse.bass as bass  # noqa: E402,F401
import concourse.mybir as mybir  # noqa: E402
import concourse.tile as tile  # noqa: E402
from concourse import bacc  # noqa: E402
from concourse import bass_utils  # noqa: E402

N, E, B, K, D, H = 8192, 256, 16, 64, 8, 3
NCORES = 8
P = 128
INV_SQRT_2 = 0.7071067811865476
SILU_SCALE = 1.0 / 0.6

F32 = mybir.dt.float32
F32R = mybir.dt.float32r
AF = mybir.ActivationFunctionType
ALU = mybir.AluOpType

TWOPI = float(2 * np.pi)
INV2PI = float(1.0 / (2 * np.pi))
MAGIC = float(3 << 22)  # fp32 round-to-nearest-int magic constant
HALFPI = float(np.pi / 2)


def _nsl(total, base=0):
    """<=512-wide slices, all >=256 (f32r matmuls run 1 cyc/row only at
    free-dim >=256). total must be a multiple of 128 and >=256."""
    out = []
    rem, o = total, 0
    while rem > 0:
        if rem >= 768 or rem <= 512:
            w = min(512, rem)
        else:  # 513..767 -> two 128-aligned pieces, both >=256
            w = (rem // 2 + 127) // 128 * 128
        out.append((base + o, w))
        o += w
        rem -= w
    return out


def build_program(slot_a, slot_b, stage=99, repeat=1):
    """Build the SPMD per-core Bass program. Static in (slot_a, slot_b)."""
    NP = slot_a + slot_b
    T = NP // P
    TA = slot_a // P

    nc = bacc.Bacc("TRN2", target_bir_lowering=False, debug=False,
                   num_devices=NCORES)

    # ---- DRAM I/O (all SBUF partition-major images; contiguous DMAs) ----
    d_hT = nc.dram_tensor("hT", [P, 2 * NP], F32R, kind="ExternalInput")
    d_h06 = nc.dram_tensor("h06", [P, T * E], F32, kind="ExternalInput")
    d_xT = nc.dram_tensor("xT", [3, NP], F32, kind="ExternalInput")
    d_kgT = nc.dram_tensor("kgT", [3, 2 * K], F32, kind="ExternalInput")
    d_kf2 = nc.dram_tensor("kf2", [P, E], F32R, kind="ExternalInput")
    d_idn = nc.dram_tensor("idn", [P, P], F32R, kind="ExternalInput")
    d_wp0 = nc.dram_tensor("wp0", [P, 4 * P], F32R, kind="ExternalInput")
    d_wp1 = nc.dram_tensor("wp1", [P, 2 * E], F32R, kind="ExternalInput")
    d_we0 = nc.dram_tensor("we0", [P, 4 * P], F32R, kind="ExternalInput")
    d_wres = nc.dram_tensor("wres", [P, 8 * H * P], F32R,
                            kind="ExternalInput")
    d_huT = nc.dram_tensor("huT", [P, 2 * NP], F32R, kind="ExternalOutput")
    d_dot = nc.dram_tensor("dot_o", [P, T * K], F32, kind="ExternalOutput")

    # two independent n-streams through the feature-major MLP chain.
    # Matmul psum writes must stay inside one 512-f32 bank, so slices are
    # plain 512-chunks relative to the half's psum tile.
    halves = [(0, 512), (512, NP - 512)]
    half_tag = ["mlpA", "mlpB"]

    def _bank_slices(ho, hw):
        out, o = [], 0
        while o < hw:
            w = min(512, hw - o)
            out.append((ho + o, w))
            o += w
        return out

    half_slices = [_bank_slices(ho, hw) for (ho, hw) in halves]
    # gather chunks each get their own psum tile (offset 0): keep all >=256
    g_chunks = [_nsl(slot_a, 0), _nsl(slot_b, slot_a)]

    with tile.TileContext(nc) as tc:
      with tc.tile_pool(name="sb", bufs=1) as sb, \
           tc.tile_pool(name="ps", bufs=1, space="PSUM") as ps:
        for _rep in range(repeat):
            # ---- inputs to SBUF, ordered by first use ----
            halfpi = sb.tile([P, 1], F32, tag="halfpi", name="halfpi")
            nc.vector.memset(halfpi[:], HALFPI)
            xT = sb.tile([3, NP], F32, tag="xT", name="xT")
            nc.sync.dma_start(xT[:], d_xT.ap())
            kg = sb.tile([3, 2, K], F32, tag="kg", name="kg")
            nc.sync.dma_start(kg.rearrange("p g k -> p (g k)")[:],
                              d_kgT.ap())
            wp0 = sb.tile([P, 4, P], F32R, tag="wp0", name="wp0")
            nc.sync.dma_start(wp0.rearrange("p q f -> p (q f)")[:],
                              d_wp0.ap())
            # hT quartered: [ei0-A, ei1-A, ei0-B, ei1-B] so z1's A-half
            # matmuls can start after ~2 quarters
            hT = sb.tile([P, 2, NP], F32R, tag="hT", name="hT")
            for (qo, qw) in ((0, 512), (512, NP - 512)):
                for ei in range(2):
                    nc.sync.dma_start(
                        hT[:, ei, qo:qo + qw],
                        d_hT.ap()[:, ei * NP + qo:ei * NP + qo + qw])
            idn = sb.tile([P, P], F32R, tag="idn", name="idn")
            nc.sync.dma_start(idn[:], d_idn.ap())
            wp1 = sb.tile([P, 2, E], F32R, tag="wp1", name="wp1")
            nc.sync.dma_start(wp1.rearrange("p q f -> p (q f)")[:],
                              d_wp1.ap())
            h06 = sb.tile([P, T, E], F32, tag="h06", name="h06")
            nc.sync.dma_start(h06.rearrange("p t e -> p (t e)")[:],
                              d_h06.ap())
            kf2 = sb.tile([P, E], F32R, tag="kf2", name="kf2")
            nc.sync.dma_start(kf2[:], d_kf2.ap())
            we0 = sb.tile([P, 4, P], F32R, tag="we0", name="we0")
            nc.sync.dma_start(we0.rearrange("p q f -> p (q f)")[:],
                              d_we0.ap())
            wres = sb.tile([P, 8 * H, P], F32R, tag="wres", name="wres")
            nc.sync.dma_start(wres.rearrange("p q f -> p (q f)")[:],
                              d_wres.ap())

            # ---- stage D: dot products + trig (atom-major) ----
            # CS_at[:, t, 0:64] = cos, [:, t, 64:128] = sin  (fused so the
            # sf matmul emits sf_r on psum rows 0:64 and sf_i on 64:128)
            CS_at = sb.tile([P, T, 2 * K], F32R, tag="CS_at", name="CS_at")
            dot_sb = sb.tile([P, T, K], F32, tag="dot_sb", name="dot_sb")
            tiles = list(range(T))
            sin_insts = []
            for p0 in range(0, T, 4):
                pk = tiles[p0:p0 + 4]
                w = len(pk) * K
                dps = ps.tile([P, 512], F32,
                              tag=("small" if (p0 // 4) % 2 == 0 else "sf"),
                              name=f"dot_{p0}")
                for j, t in enumerate(pk):
                    g = 0 if t < TA else 1
                    nc.tensor.matmul(
                        dps[:, j * K:(j + 1) * K],
                        lhsT=xT[0:3, t * P:(t + 1) * P],
                        rhs=kg[0:3, g, :],
                        start=True, stop=True)
                fl = slice(p0 * K, p0 * K + w)  # flattened (T,K) slice
                dflat = dot_sb.rearrange("p t k -> p (t k)")
                t1 = sb.tile([P, 512], F32, tag="t1", bufs=2,
                             name=f"t1_{p0}")
                nc.vector.tensor_scalar(t1[:, :w], dps[:, :w], INV2PI,
                                        MAGIC, ALU.mult, ALU.add)
                k1 = sb.tile([P, 512], F32, tag="k1", bufs=2,
                             name=f"k1_{p0}")
                nc.vector.tensor_scalar(k1[:, :w], t1[:, :w], MAGIC, None,
                                        ALU.subtract)
                rr = sb.tile([P, 512], F32, tag="rr", bufs=2,
                             name=f"rr_{p0}")
                nc.vector.scalar_tensor_tensor(rr[:, :w], k1[:, :w],
                                               -TWOPI, dps[:, :w],
                                               ALU.mult, ALU.add)
                kc = sb.tile([P, 512], F32, tag="kc", bufs=2,
                             name=f"kc_{p0}")
                nc.vector.tensor_scalar(kc[:, :w], rr[:, :w], HALFPI, None,
                                        ALU.is_gt)
                rc = sb.tile([P, 512], F32, tag="rc", bufs=2,
                             name=f"rc_{p0}")
                nc.vector.scalar_tensor_tensor(rc[:, :w], kc[:, :w],
                                               -TWOPI, rr[:, :w],
                                               ALU.mult, ALU.add)
                sin_insts.append(nc.scalar.activation(
                    CS_at[:, p0:p0 + len(pk), K:2 * K],
                    rr[:, :w].rearrange("p (t k) -> p t k", k=K), AF.Sin))
                sin_insts.append(nc.scalar.activation(
                    CS_at[:, p0:p0 + len(pk), 0:K],
                    rc[:, :w].rearrange("p (t k) -> p t k", k=K),
                    AF.Sin, bias=halfpi[:]))
                nc.vector.tensor_copy(dflat[:, fl], dps[:, :w])
            nc.sync.dma_start(d_dot.ap(),
                              dot_sb.rearrange("p t k -> p (t k)")[:])

            # ---- stage Z1: z1 = hT @ Wp0 (feature-major), y1 = silu ----
            y1T = sb.tile([P, 2, NP], F32R, tag="y1T", name="y1T")
            for hf, (ho, hw) in enumerate(halves):
                for eo in range(2):
                    z1p = ps.tile([P, hw], F32, tag=half_tag[hf], bufs=2,
                                  name=f"z1_{hf}{eo}")
                    for ei in range(2):
                        for (o, w) in half_slices[hf]:
                            nc.tensor.matmul(z1p[:, o - ho:o - ho + w],
                                             lhsT=wp0[:, ei * 2 + eo, :],
                                             rhs=hT[:, ei, o:o + w],
                                             start=(ei == 0),
                                             stop=(ei == 1))
                    _silu1 = nc.scalar.activation(y1T[:, eo, ho:ho + hw],
                                                  z1p[:], AF.Silu)
                    # keep all Sin ACTs before any Silu: one table switch
                    tile.add_dep_helper(_silu1.ins, sin_insts[-1].ins,
                                        sync=False,
                                        reason="act table grouping")

            # ---- stage T: CST = transpose(CS_at) ----
            # CST rows 0:64 = cos^T, rows 64:128 = sin^T (k-major)
            CST = sb.tile([P, NP], F32R, tag="CST", name="CST")
            for p0 in range(0, T, 4):
                pk = tiles[p0:p0 + 4]
                w = len(pk) * P
                tp = ps.tile([P, 512], F32R, tag="small", name=f"tp_{p0}")
                for j, t in enumerate(pk):
                    nc.tensor.transpose(tp[:, j * P:(j + 1) * P],
                                        CS_at[:, t, :], idn[:])
                nc.vector.tensor_copy(CST[:, p0 * P:p0 * P + w], tp[:, :w])

            # ---- stage Z2 + hres + sf ----
            hres = sb.tile([P, T, E], F32R, tag="hres", name="hres")
            sfp = ps.tile([P, 512], F32, tag="sf", name="sfp")
            h06f = h06.rearrange("p t e -> p (t e)")
            hresf = hres.rearrange("p t e -> p (t e)")
            hu0T = sb.tile([P, 2, NP], F32R, tag="hu0T", name="hu0T")
            filt = [None, None]

            def emit_filt_gather(g):
                fg = sb.tile([P, E], F32R, tag=f"filt{g}", name=f"filt_{g}")
                nc.vector.tensor_mul(fg[:], sfp[:, g * E:(g + 1) * E],
                                     kf2[:])
                filt[g] = fg
                for eo in range(2):
                    for gi, (o, w) in enumerate(g_chunks[g]):
                        gp = ps.tile([P, 512], F32,
                                     tag=half_tag[(eo + gi) % 2], bufs=2,
                                     name=f"g_{g}{eo}{o}")
                        nc.tensor.matmul(gp[:, :w],
                                         lhsT=fg[:, eo * P:(eo + 1) * P],
                                         rhs=CST[:, o:o + w],
                                         start=True, stop=True)
                        nc.vector.tensor_copy(hu0T[:, eo, o:o + w],
                                              gp[:, :w])

            z2packs = [tiles[p0:p0 + 2] for p0 in range(0, T, 2)]
            sf_last = [None]
            for pk in z2packs:
                p0 = pk[0]
                w = len(pk) * E
                zp = ps.tile([P, 512], F32, tag="mlpA", bufs=2,
                             name=f"z2_{p0}")
                for j, t in enumerate(pk):
                    for ei in range(2):
                        nc.tensor.matmul(
                            zp[:, j * E:(j + 1) * E],
                            lhsT=y1T[:, ei, t * P:(t + 1) * P],
                            rhs=wp1[:, ei, :],
                            start=(ei == 0), stop=(ei == 1))
                t2 = sb.tile([P, 512], F32, tag="t2", bufs=2,
                             name=f"t2_{p0}")
                nc.scalar.activation(t2[:, :w], zp[:, :w], AF.Silu)
                fl = slice(p0 * E, p0 * E + w)
                nc.vector.tensor_add(hresf[:, fl], t2[:, :w], h06f[:, fl])
                for t in pk:
                    g = 0 if t < TA else 1
                    first = t in (0, TA)
                    last = t in (TA - 1, T - 1)
                    # sf[:, g*256:]: rows 0:64 = sf_r, rows 64:128 = sf_i
                    _sfmm = nc.tensor.matmul(sfp[:, g * E:(g + 1) * E],
                                             lhsT=CS_at[:, t, :],
                                             rhs=hres[:, t, :],
                                             start=first, stop=last)
                    if first and g == 1 and sf_last[0] is not None:
                        # B's accumulation group must open after A's closes
                        # (one psum bank holds both zero regions)
                        tile.add_dep_helper(_sfmm.ins, sf_last[0].ins,
                                            sync=False,
                                            reason="sf bank group order")
                    if last:
                        sf_last[0] = _sfmm
                        emit_filt_gather(g)

            # ---- e0 dense + 3 residual layers (feature-major, 2 streams) --
            def mm_set(lhs_w, qbase, rhs, hf, eo, name):
                ho, hw = halves[hf]
                pp = ps.tile([P, hw], F32, tag=half_tag[hf], bufs=2,
                             name=f"{name}_{hf}{eo}")
                for ei in range(2):
                    for (o, w) in half_slices[hf]:
                        nc.tensor.matmul(pp[:, o - ho:o - ho + w],
                                         lhsT=lhs_w[:, qbase + ei * 2 + eo,
                                                    :],
                                         rhs=rhs[:, ei, o:o + w],
                                         start=(ei == 0), stop=(ei == 1))
                return pp

            x0 = sb.tile([P, 2, NP], F32R, tag="xhat", bufs=2, name="x0")
            for hf, (ho, hw) in enumerate(halves):
                for eo in range(2):
                    pp = mm_set(we0, 0, hu0T, hf, eo, "e0")
                    nc.scalar.activation(x0[:, eo, ho:ho + hw], pp[:],
                                         AF.Silu)

            xcur = x0
            for i in range(H):
                gamma = float(math.sqrt(2.0) ** i)
                ta = sb.tile([P, 2, NP], F32R, tag="tA", bufs=2,
                             name=f"ta_{i}")
                for hf, (ho, hw) in enumerate(halves):
                    for eo in range(2):
                        pp = mm_set(wres, 8 * i, xcur, hf, eo, f"ra{i}")
                        nc.scalar.activation(ta[:, eo, ho:ho + hw], pp[:],
                                             AF.Silu)
                xn = sb.tile([P, 2, NP], F32R, tag="xhat", bufs=2,
                             name=f"x{i + 1}")
                last = i == H - 1
                for hf, (ho, hw) in enumerate(halves):
                    for eo in range(2):
                        pp = mm_set(wres, 8 * i + 4, ta, hf, eo, f"rb{i}")
                        tb = sb.tile([P, 640], F32, tag="tB", bufs=3,
                                     name=f"tb_{i}{hf}{eo}")
                        nc.scalar.activation(tb[:, :hw], pp[:], AF.Silu)
                        nc.vector.scalar_tensor_tensor(
                            xn[:, eo, ho:ho + hw], tb[:, :hw], gamma,
                            xcur[:, eo, ho:ho + hw], ALU.mult, ALU.add)
                        if last:
                            nc.sync.dma_start(
                                d_huT.ap()[:, eo * NP + ho:
                                           eo * NP + ho + hw],
                                xn[:, eo, ho:ho + hw])
                xcur = xn

    nc.compile()
    return nc


_PROG_CACHE = {}


def _get_program(slot_a, slot_b):
    key = (slot_a, slot_b)
    if key not in _PROG_CACHE:
        _PROG_CACHE[key] = build_program(slot_a, slot_b)
    return _PROG_CACHE[key]


def _preprocess(h, x, batch_seg, kk, W_down, W_up, W_pre, W_e0, W_res):
    """Host-side sharding + weight folding. Returns (in_maps, meta)."""
    c = SILU_SCALE
    s = INV_SQRT_2

    bs = np.asarray(batch_seg).astype(np.int64)
    offs = np.searchsorted(bs, np.arange(B + 1))
    sizes = np.diff(offs)

    # pair graphs: i-th largest with i-th smallest
    order = np.argsort(-sizes, kind="stable")
    pairs = [(int(order[i]), int(order[2 * NCORES - 1 - i]))
             for i in range(NCORES)]
    big = [p[0] for p in pairs]
    small = [p[1] for p in pairs]
    slot_a = int(math.ceil(max(sizes[g] for g in big) / P) * P)
    slot_b = int(math.ceil(max(max(sizes[g] for g in small), 1) / P) * P)
    slot_a = max(slot_a, 256)
    slot_b = max(slot_b, 256)
    NP = slot_a + slot_b
    T = NP // P

    # ---- shared weights (folded scales) ----
    beta = [0.6 * (math.sqrt(2.0) ** i) for i in range(H + 1)]
    wp0 = np.ascontiguousarray(W_pre[0].T)                   # [ei, eo]
    wp1 = np.ascontiguousarray((c * W_pre[1]).T)
    we0 = np.ascontiguousarray(W_e0.T)
    wres_l = []
    for i in range(H):
        wres_l.append(np.ascontiguousarray((W_res[i, 0] / beta[i]).T))
        wres_l.append(np.ascontiguousarray((c * W_res[i, 1]).T))

    def blocks4(w):  # [256,256] -> [4,128,128] (q = ei*2+eo)
        return np.ascontiguousarray(
            w.reshape(2, P, 2, P).transpose(0, 2, 1, 3).reshape(4, P, P))

    wp0_b = blocks4(wp0)
    we0_b = blocks4(we0)
    wres_b = np.empty((8 * H, P, P), np.float32)
    for i in range(H):
        wres_b[8 * i:8 * i + 4] = blocks4(wres_l[2 * i])
        wres_b[8 * i + 4:8 * i + 8] = blocks4(wres_l[2 * i + 1])
    wp1_b = np.ascontiguousarray(wp1.reshape(2, P, E))

    def pmajor(a):  # [Q, P, F] -> SBUF image [P, Q*F]
        q, p, f = a.shape
        return np.ascontiguousarray(
            a.transpose(1, 0, 2).reshape(p, q * f)).astype(np.float32)

    kfilter = (W_up @ W_down).T                              # [K, E]
    kf_dev = (0.01 * c * s) * kfilter
    kf2 = np.ascontiguousarray(np.concatenate([kf_dev, kf_dev], axis=0)
                               ).astype(np.float32)          # [128, E]
    idn = np.eye(P, dtype=np.float32)

    shared = {
        "kf2": kf2, "idn": idn, "wp0": pmajor(wp0_b),
        "wp1": pmajor(wp1_b), "we0": pmajor(we0_b),
        "wres": pmajor(wres_b),
    }

    in_maps = []
    meta = []
    for cix in range(NCORES):
        gA, gB = pairs[cix]
        szA, szB = int(sizes[gA]), int(sizes[gB])
        hpad = np.zeros((NP, E), np.float32)
        xpad = np.zeros((NP, 3), np.float32)
        hpad[0:szA] = h[offs[gA]:offs[gA] + szA]
        xpad[0:szA] = x[offs[gA]:offs[gA] + szA]
        hpad[slot_a:slot_a + szB] = h[offs[gB]:offs[gB] + szB]
        xpad[slot_a:slot_a + szB] = x[offs[gB]:offs[gB] + szB]
        m = dict(shared)
        m["hT"] = pmajor(hpad.T.reshape(2, P, NP))
        m["h06"] = pmajor((hpad / c).reshape(T, P, E))
        m["xT"] = np.ascontiguousarray(xpad.T)
        m["kgT"] = np.ascontiguousarray(
            np.stack([kk[gA].T, kk[gB].T], axis=1).reshape(3, 2 * K)
        ).astype(np.float32)
        in_maps.append(m)
        meta.append((gA, gB, szA, szB))

    return in_maps, meta, (slot_a, slot_b, offs, sizes, beta)


def kernel(h, x, k, num_batch, batch_seg, W_down, W_up, W_pre, W_e0, W_res,
           _want_trace=False):
    h = np.asarray(h, np.float32)
    x = np.asarray(x, np.float32)
    kk = np.asarray(k, np.float32)
    W_down = np.asarray(W_down, np.float32)
    W_up = np.asarray(W_up, np.float32)
    W_pre = np.asarray(W_pre, np.float32)
    W_e0 = np.asarray(W_e0, np.float32)
    W_res = np.asarray(W_res, np.float32)

    in_maps, meta, (slot_a, slot_b, offs, sizes, beta) = _preprocess(
        h, x, batch_seg, kk, W_down, W_up, W_pre, W_e0, W_res)
    NP = slot_a + slot_b
    T = NP // P

    nc = _get_program(slot_a, slot_b)
    res = bass_utils.run_bass_kernel_spmd(
        nc, in_maps, core_ids=list(range(NCORES)), trace=_want_trace)

    hu = np.zeros((N, E), np.float32)
    dot = np.zeros((N, K), np.float32)
    out_scale = np.float32(1.0 / beta[H])
    for cix in range(NCORES):
        gA, gB, szA, szB = meta[cix]
        r = res.results[cix]
        # huT image [P, 2, NP] -> [2, P, NP] -> [256, NP] -> [NP, 256]
        huT = np.asarray(r["huT"]).reshape(P, 2, NP).transpose(1, 0, 2)
        hu_pad = huT.reshape(2 * P, NP).T  # [NP, E]
        # dot image [P, T, K] -> [T, P, K] -> [NP, K]
        dpad = np.asarray(r["dot_o"]).reshape(P, T, K).transpose(
            1, 0, 2).reshape(NP, K)
        hu[offs[gA]:offs[gA] + szA] = hu_pad[0:szA]
        hu[offs[gB]:offs[gB] + szB] = hu_pad[slot_a:slot_a + szB]
        dot[offs[gA]:offs[gA] + szA] = dpad[0:szA]
        dot[offs[gB]:offs[gB] + szB] = dpad[slot_a:slot_a + szB]
    hu *= out_scale

    if _want_trace:
        kernel._last_results = res  # stash for profiling harnesses
    return (hu, dot, np.float32(1.0))


# revision 35
# speedup vs baseline: 1.0022x; 1.0022x over previous
"""Trainium2 Bass kernel for nn_EwaldBlock (gnn_message_passing).

Sharding: data-parallel over graphs. 16 graphs -> 8 cores, 2 whole graphs per
core (paired large+small, padded to static 128-aligned slots), so the
segment_sum stays core-local. Small weights are replicated.

Per-core device program (feature-major activations [E=2x128 part-tiles, n]):
  dot (PE, fp32) -> sin/cos via range-reduced ACT Sin (fused C|S layout)
  z1 = h @ Wp0.T (f32r), silu; z2 atom-major via activation-stationary
  matmuls, silu; hres = 0.6*h + silu(z2)   (GemNet 1/0.6 & 1/sqrt2 scales
  folded into host-preprocessed weights)
  sf (segment outer-product sums): one K=128 matmul per 128-atom tile emits
  sf_r (psum rows 0:64) and sf_i (rows 64:128) at once
  gather-back: one K=128 matmul per chunk = filt_r.T@cosT + filt_i.T@sinT
  e0 dense + 3 residual layers, feature-major f32r matmuls, pipelined as two
  independent n-streams so ACT(silu) and PE overlap across layers.
"""

import os
import sys
import math
import numpy as np

sys.path.insert(0, "/opt/trn_rl_repo")

import concourse.bass as bass  # noqa: E402,F401
import concourse.mybir as mybir  # noqa: E402
import concourse.tile as tile  # noqa: E402
from concourse import bacc  # noqa: E402
from concourse import bass_utils  # noqa: E402

N, E, B, K, D, H = 8192, 256, 16, 64, 8, 3
NCORES = 8
P = 128
INV_SQRT_2 = 0.7071067811865476
SILU_SCALE = 1.0 / 0.6

F32 = mybir.dt.float32
F32R = mybir.dt.float32r
AF = mybir.ActivationFunctionType
ALU = mybir.AluOpType

TWOPI = float(2 * np.pi)
INV2PI = float(1.0 / (2 * np.pi))
MAGIC = float(3 << 22)  # fp32 round-to-nearest-int magic constant
HALFPI = float(np.pi / 2)


def _nsl(total, base=0):
    """<=512-wide slices, all >=256 (f32r matmuls run 1 cyc/row only at
    free-dim >=256). total must be a multiple of 128 and >=256."""
    out = []
    rem, o = total, 0
    while rem > 0:
        if rem >= 768 or rem <= 512:
            w = min(512, rem)
        else:  # 513..767 -> two 128-aligned pieces, both >=256
            w = (rem // 2 + 127) // 128 * 128
        out.append((base + o, w))
        o += w
        rem -= w
    return out


def build_program(slot_a, slot_b, stage=99, repeat=1):
    """Build the SPMD per-core Bass program. Static in (slot_a, slot_b)."""
    NP = slot_a + slot_b
    T = NP // P
    TA = slot_a // P

    nc = bacc.Bacc("TRN2", target_bir_lowering=False, debug=False,
                   num_devices=NCORES)

    # ---- DRAM I/O (all SBUF partition-major images; contiguous DMAs) ----
    d_hT = nc.dram_tensor("hT", [P, 2 * NP], F32R, kind="ExternalInput")
    d_h06 = nc.dram_tensor("h06", [P, T * E], F32, kind="ExternalInput")
    d_xT = nc.dram_tensor("xT", [3, NP], F32, kind="ExternalInput")
    d_kgT = nc.dram_tensor("kgT", [3, 2 * K], F32, kind="ExternalInput")
    d_kf2 = nc.dram_tensor("kf2", [P, E], F32R, kind="ExternalInput")
    d_idn = nc.dram_tensor("idn", [P, P], F32R, kind="ExternalInput")
    d_wp0 = nc.dram_tensor("wp0", [P, 4 * P], F32R, kind="ExternalInput")
    d_wp1 = nc.dram_tensor("wp1", [P, 2 * E], F32R, kind="ExternalInput")
    d_we0 = nc.dram_tensor("we0", [P, 4 * P], F32R, kind="ExternalInput")
    d_wres = nc.dram_tensor("wres", [P, 8 * H * P], F32R,
                            kind="ExternalInput")
    d_huT = nc.dram_tensor("huT", [P, 2 * NP], F32R, kind="ExternalOutput")
    d_dot = nc.dram_tensor("dot_o", [P, T * K], F32, kind="ExternalOutput")

    # two independent n-streams through the feature-major MLP chain.
    # Matmul psum writes must stay inside one 512-f32 bank, so slices are
    # plain 512-chunks relative to the half's psum tile.
    halves = [(0, 512), (512, NP - 512)]
    half_tag = ["mlpA", "mlpB"]

    def _bank_slices(ho, hw):
        out, o = [], 0
        while o < hw:
            w = min(512, hw - o)
            out.append((ho + o, w))
            o += w
        return out

    half_slices = [_bank_slices(ho, hw) for (ho, hw) in halves]
    # gather chunks each get their own psum tile (offset 0): keep all >=256
    g_chunks = [_nsl(slot_a, 0), _nsl(slot_b, slot_a)]

    with tile.TileContext(nc) as tc:
      with tc.tile_pool(name="sb", bufs=1) as sb, \
           tc.tile_pool(name="ps", bufs=1, space="PSUM") as ps:
        for _rep in range(repeat):
            # ---- inputs to SBUF, ordered by first use ----
            halfpi = sb.tile([P, 1], F32, tag="halfpi", name="halfpi")
            nc.vector.memset(halfpi[:], HALFPI)
            xT = sb.tile([3, NP], F32, tag="xT", name="xT")
            nc.sync.dma_start(xT[:], d_xT.ap())
            kg = sb.tile([3, 2, K], F32, tag="kg", name="kg")
            nc.sync.dma_start(kg.rearrange("p g k -> p (g k)")[:],
                              d_kgT.ap())
            wp0 = sb.tile([P, 4, P], F32R, tag="wp0", name="wp0")
            nc.sync.dma_start(wp0.rearrange("p q f -> p (q f)")[:],
                              d_wp0.ap())
            # hT quartered: [ei0-A, ei1-A, ei0-B, ei1-B] so z1's A-half
            # matmuls can start after ~2 quarters
            hT = sb.tile([P, 2, NP], F32R, tag="hT", name="hT")
            for (qo, qw) in ((0, 512), (512, NP - 512)):
                for ei in range(2):
                    nc.sync.dma_start(
                        hT[:, ei, qo:qo + qw],
                        d_hT.ap()[:, ei * NP + qo:ei * NP + qo + qw])
            idn = sb.tile([P, P], F32R, tag="idn", name="idn")
            nc.sync.dma_start(idn[:], d_idn.ap())
            wp1 = sb.tile([P, 2, E], F32R, tag="wp1", name="wp1")
            nc.sync.dma_start(wp1.rearrange("p q f -> p (q f)")[:],
                              d_wp1.ap())
            h06 = sb.tile([P, T, E], F32, tag="h06", name="h06")
            nc.sync.dma_start(h06.rearrange("p t e -> p (t e)")[:],
                              d_h06.ap())
            kf2 = sb.tile([P, E], F32R, tag="kf2", name="kf2")
            nc.sync.dma_start(kf2[:], d_kf2.ap())
            we0 = sb.tile([P, 4, P], F32R, tag="we0", name="we0")
            nc.sync.dma_start(we0.rearrange("p q f -> p (q f)")[:],
                              d_we0.ap())
            wres = sb.tile([P, 8 * H, P], F32R, tag="wres", name="wres")
            nc.sync.dma_start(wres.rearrange("p q f -> p (q f)")[:],
                              d_wres.ap())

            # ---- stage D: dot products + trig (atom-major) ----
            # CS_at[:, t, 0:64] = cos, [:, t, 64:128] = sin  (fused so the
            # sf matmul emits sf_r on psum rows 0:64 and sf_i on 64:128)
            CS_at = sb.tile([P, T, 2 * K], F32R, tag="CS_at", name="CS_at")
            dot_sb = sb.tile([P, T, K], F32, tag="dot_sb", name="dot_sb")
            tiles = list(range(T))
            sin_insts = []
            for p0 in range(0, T, 4):
                pk = tiles[p0:p0 + 4]
                w = len(pk) * K
                dps = ps.tile([P, 512], F32,
                              tag=("small" if (p0 // 4) % 2 == 0 else "sf"),
                              name=f"dot_{p0}")
                for j, t in enumerate(pk):
                    g = 0 if t < TA else 1
                    nc.tensor.matmul(
                        dps[:, j * K:(j + 1) * K],
                        lhsT=xT[0:3, t * P:(t + 1) * P],
                        rhs=kg[0:3, g, :],
                        start=True, stop=True)
                fl = slice(p0 * K, p0 * K + w)  # flattened (T,K) slice
                dflat = dot_sb.rearrange("p t k -> p (t k)")
                t1 = sb.tile([P, 512], F32, tag="t1", bufs=2,
                             name=f"t1_{p0}")
                nc.vector.tensor_scalar(t1[:, :w], dps[:, :w], INV2PI,
                                        MAGIC, ALU.mult, ALU.add)
                k1 = sb.tile([P, 512], F32, tag="k1", bufs=2,
                             name=f"k1_{p0}")
                nc.vector.tensor_scalar(k1[:, :w], t1[:, :w], MAGIC, None,
                                        ALU.subtract)
                rr = sb.tile([P, 512], F32, tag="rr", bufs=2,
                             name=f"rr_{p0}")
                nc.vector.scalar_tensor_tensor(rr[:, :w], k1[:, :w],
                                               -TWOPI, dps[:, :w],
                                               ALU.mult, ALU.add)
                kc = sb.tile([P, 512], F32, tag="kc", bufs=2,
                             name=f"kc_{p0}")
                nc.vector.tensor_scalar(kc[:, :w], rr[:, :w], HALFPI, None,
                                        ALU.is_gt)
                rc = sb.tile([P, 512], F32, tag="rc", bufs=2,
                             name=f"rc_{p0}")
                nc.vector.scalar_tensor_tensor(rc[:, :w], kc[:, :w],
                                               -TWOPI, rr[:, :w],
                                               ALU.mult, ALU.add)
                sin_insts.append(nc.scalar.activation(
                    CS_at[:, p0:p0 + len(pk), K:2 * K],
                    rr[:, :w].rearrange("p (t k) -> p t k", k=K), AF.Sin))
                sin_insts.append(nc.scalar.activation(
                    CS_at[:, p0:p0 + len(pk), 0:K],
                    rc[:, :w].rearrange("p (t k) -> p t k", k=K),
                    AF.Sin, bias=halfpi[:]))
                nc.vector.tensor_copy(dflat[:, fl], dps[:, :w])
            nc.sync.dma_start(d_dot.ap(),
                              dot_sb.rearrange("p t k -> p (t k)")[:])

            # ---- stage Z1: z1 = hT @ Wp0 (feature-major), y1 = silu ----
            y1T = sb.tile([P, 2, NP], F32R, tag="y1T", name="y1T")
            for hf, (ho, hw) in enumerate(halves):
                for eo in range(2):
                    z1p = ps.tile([P, hw], F32, tag=half_tag[hf], bufs=2,
                                  name=f"z1_{hf}{eo}")
                    for ei in range(2):
                        for (o, w) in half_slices[hf]:
                            nc.tensor.matmul(z1p[:, o - ho:o - ho + w],
                                             lhsT=wp0[:, ei * 2 + eo, :],
                                             rhs=hT[:, ei, o:o + w],
                                             start=(ei == 0),
                                             stop=(ei == 1))
                    _silu1 = nc.scalar.activation(y1T[:, eo, ho:ho + hw],
                                                  z1p[:], AF.Silu)
                    # keep all Sin ACTs before any Silu: one table switch
                    tile.add_dep_helper(_silu1.ins, sin_insts[-1].ins,
                                        sync=False,
                                        reason="act table grouping")

            # ---- stage T: CST = transpose(CS_at) ----
            # CST rows 0:64 = cos^T, rows 64:128 = sin^T (k-major)
            CST = sb.tile([P, NP], F32R, tag="CST", name="CST")
            for p0 in range(0, T, 4):
                pk = tiles[p0:p0 + 4]
                w = len(pk) * P
                tp = ps.tile([P, 512], F32R, tag="small", name=f"tp_{p0}")
                for j, t in enumerate(pk):
                    nc.tensor.transpose(tp[:, j * P:(j + 1) * P],
                                        CS_at[:, t, :], idn[:])
                nc.vector.tensor_copy(CST[:, p0 * P:p0 * P + w], tp[:, :w])

            # ---- stage Z2 + hres + sf ----
            hres = sb.tile([P, T, E], F32R, tag="hres", name="hres")
            sfp = ps.tile([P, 512], F32, tag="sf", name="sfp")
            h06f = h06.rearrange("p t e -> p (t e)")
            hresf = hres.rearrange("p t e -> p (t e)")
            hu0T = sb.tile([P, 2, NP], F32R, tag="hu0T", name="hu0T")
            filt = [None, None]

            def emit_filt_gather(g):
                fg = sb.tile([P, E], F32R, tag=f"filt{g}", name=f"filt_{g}")
                nc.vector.tensor_mul(fg[:], sfp[:, g * E:(g + 1) * E],
                                     kf2[:])
                filt[g] = fg
                for eo in range(2):
                    for gi, (o, w) in enumerate(g_chunks[g]):
                        gp = ps.tile([P, 512], F32,
                                     tag=half_tag[(eo + gi) % 2], bufs=2,
                                     name=f"g_{g}{eo}{o}")
                        nc.tensor.matmul(gp[:, :w],
                                         lhsT=fg[:, eo * P:(eo + 1) * P],
                                         rhs=CST[:, o:o + w],
                                         start=True, stop=True)
                        nc.vector.tensor_copy(hu0T[:, eo, o:o + w],
                                              gp[:, :w])

            z2packs = [tiles[p0:p0 + 4] for p0 in range(0, T, 4)]
            sf_last = [None]
            for pk in z2packs:
                p0 = pk[0]
                w = len(pk) * E
                zp = ps.tile([P, 1024], F32, tag="mlpB", bufs=2,
                             name=f"z2_{p0}")
                for j, t in enumerate(pk):
                    for ei in range(2):
                        nc.tensor.matmul(
                            zp[:, j * E:(j + 1) * E],
                            lhsT=y1T[:, ei, t * P:(t + 1) * P],
                            rhs=wp1[:, ei, :],
                            start=(ei == 0), stop=(ei == 1))
                t2 = sb.tile([P, 1024], F32, tag="t2", bufs=2,
                             name=f"t2_{p0}")
                nc.scalar.activation(t2[:, :w], zp[:, :w], AF.Silu)
                fl = slice(p0 * E, p0 * E + w)
                nc.vector.tensor_add(hresf[:, fl], t2[:, :w], h06f[:, fl])
                for t in pk:
                    g = 0 if t < TA else 1
                    first = t in (0, TA)
                    last = t in (TA - 1, T - 1)
                    # sf[:, g*256:]: rows 0:64 = sf_r, rows 64:128 = sf_i
                    _sfmm = nc.tensor.matmul(sfp[:, g * E:(g + 1) * E],
                                             lhsT=CS_at[:, t, :],
                                             rhs=hres[:, t, :],
                                             start=first, stop=last)
                    if first and g == 1 and sf_last[0] is not None:
                        # B's accumulation group must open after A's closes
                        # (one psum bank holds both zero regions)
                        tile.add_dep_helper(_sfmm.ins, sf_last[0].ins,
                                            sync=False,
                                            reason="sf bank group order")
                    if last:
                        sf_last[0] = _sfmm
                        emit_filt_gather(g)

            # ---- e0 dense + 3 residual layers (feature-major, 2 streams) --
            def mm_set(lhs_w, qbase, rhs, hf, eo, name):
                ho, hw = halves[hf]
                pp = ps.tile([P, hw], F32, tag=half_tag[hf], bufs=2,
                             name=f"{name}_{hf}{eo}")
                for ei in range(2):
                    for (o, w) in half_slices[hf]:
                        nc.tensor.matmul(pp[:, o - ho:o - ho + w],
                                         lhsT=lhs_w[:, qbase + ei * 2 + eo,
                                                    :],
                                         rhs=rhs[:, ei, o:o + w],
                                         start=(ei == 0), stop=(ei == 1))
                return pp

            x0 = sb.tile([P, 2, NP], F32R, tag="xhat", bufs=2, name="x0")
            for hf, (ho, hw) in enumerate(halves):
                for eo in range(2):
                    pp = mm_set(we0, 0, hu0T, hf, eo, "e0")
                    nc.scalar.activation(x0[:, eo, ho:ho + hw], pp[:],
                                         AF.Silu)

            xcur = x0
            for i in range(H):
                gamma = float(math.sqrt(2.0) ** i)
                ta = sb.tile([P, 2, NP], F32R, tag="tA", bufs=2,
                             name=f"ta_{i}")
                for hf, (ho, hw) in enumerate(halves):
                    for eo in range(2):
                        pp = mm_set(wres, 8 * i, xcur, hf, eo, f"ra{i}")
                        nc.scalar.activation(ta[:, eo, ho:ho + hw], pp[:],
                                             AF.Silu)
                xn = sb.tile([P, 2, NP], F32R, tag="xhat", bufs=2,
                             name=f"x{i + 1}")
                last = i == H - 1
                for hf, (ho, hw) in enumerate(halves):
                    for eo in range(2):
                        pp = mm_set(wres, 8 * i + 4, ta, hf, eo, f"rb{i}")
                        tb = sb.tile([P, 640], F32, tag="tB", bufs=3,
                                     name=f"tb_{i}{hf}{eo}")
                        nc.scalar.activation(tb[:, :hw], pp[:], AF.Silu)
                        nc.vector.scalar_tensor_tensor(
                            xn[:, eo, ho:ho + hw], tb[:, :hw], gamma,
                            xcur[:, eo, ho:ho + hw], ALU.mult, ALU.add)
                        if last:
                            nc.sync.dma_start(
                                d_huT.ap()[:, eo * NP + ho:
                                           eo * NP + ho + hw],
                                xn[:, eo, ho:ho + hw])
                xcur = xn

    nc.compile()
    return nc


_PROG_CACHE = {}


def _get_program(slot_a, slot_b):
    key = (slot_a, slot_b)
    if key not in _PROG_CACHE:
        _PROG_CACHE[key] = build_program(slot_a, slot_b)
    return _PROG_CACHE[key]


def _preprocess(h, x, batch_seg, kk, W_down, W_up, W_pre, W_e0, W_res):
    """Host-side sharding + weight folding. Returns (in_maps, meta)."""
    c = SILU_SCALE
    s = INV_SQRT_2

    bs = np.asarray(batch_seg).astype(np.int64)
    offs = np.searchsorted(bs, np.arange(B + 1))
    sizes = np.diff(offs)

    # pair graphs: i-th largest with i-th smallest
    order = np.argsort(-sizes, kind="stable")
    pairs = [(int(order[i]), int(order[2 * NCORES - 1 - i]))
             for i in range(NCORES)]
    big = [p[0] for p in pairs]
    small = [p[1] for p in pairs]
    slot_a = int(math.ceil(max(sizes[g] for g in big) / P) * P)
    slot_b = int(math.ceil(max(max(sizes[g] for g in small), 1) / P) * P)
    slot_a = max(slot_a, 256)
    slot_b = max(slot_b, 256)
    NP = slot_a + slot_b
    T = NP // P

    # ---- shared weights (folded scales) ----
    beta = [0.6 * (math.sqrt(2.0) ** i) for i in range(H + 1)]
    wp0 = np.ascontiguousarray(W_pre[0].T)                   # [ei, eo]
    wp1 = np.ascontiguousarray((c * W_pre[1]).T)
    we0 = np.ascontiguousarray(W_e0.T)
    wres_l = []
    for i in range(H):
        wres_l.append(np.ascontiguousarray((W_res[i, 0] / beta[i]).T))
        wres_l.append(np.ascontiguousarray((c * W_res[i, 1]).T))

    def blocks4(w):  # [256,256] -> [4,128,128] (q = ei*2+eo)
        return np.ascontiguousarray(
            w.reshape(2, P, 2, P).transpose(0, 2, 1, 3).reshape(4, P, P))

    wp0_b = blocks4(wp0)
    we0_b = blocks4(we0)
    wres_b = np.empty((8 * H, P, P), np.float32)
    for i in range(H):
        wres_b[8 * i:8 * i + 4] = blocks4(wres_l[2 * i])
        wres_b[8 * i + 4:8 * i + 8] = blocks4(wres_l[2 * i + 1])
    wp1_b = np.ascontiguousarray(wp1.reshape(2, P, E))

    def pmajor(a):  # [Q, P, F] -> SBUF image [P, Q*F]
        q, p, f = a.shape
        return np.ascontiguousarray(
            a.transpose(1, 0, 2).reshape(p, q * f)).astype(np.float32)

    kfilter = (W_up @ W_down).T                              # [K, E]
    kf_dev = (0.01 * c * s) * kfilter
    kf2 = np.ascontiguousarray(np.concatenate([kf_dev, kf_dev], axis=0)
                               ).astype(np.float32)          # [128, E]
    idn = np.eye(P, dtype=np.float32)

    shared = {
        "kf2": kf2, "idn": idn, "wp0": pmajor(wp0_b),
        "wp1": pmajor(wp1_b), "we0": pmajor(we0_b),
        "wres": pmajor(wres_b),
    }

    in_maps = []
    meta = []
    for cix in range(NCORES):
        gA, gB = pairs[cix]
        szA, szB = int(sizes[gA]), int(sizes[gB])
        hpad = np.zeros((NP, E), np.float32)
        xpad = np.zeros((NP, 3), np.float32)
        hpad[0:szA] = h[offs[gA]:offs[gA] + szA]
        xpad[0:szA] = x[offs[gA]:offs[gA] + szA]
        hpad[slot_a:slot_a + szB] = h[offs[gB]:offs[gB] + szB]
        xpad[slot_a:slot_a + szB] = x[offs[gB]:offs[gB] + szB]
        m = dict(shared)
        m["hT"] = pmajor(hpad.T.reshape(2, P, NP))
        m["h06"] = pmajor((hpad / c).reshape(T, P, E))
        m["xT"] = np.ascontiguousarray(xpad.T)
        m["kgT"] = np.ascontiguousarray(
            np.stack([kk[gA].T, kk[gB].T], axis=1).reshape(3, 2 * K)
        ).astype(np.float32)
        in_maps.append(m)
        meta.append((gA, gB, szA, szB))

    return in_maps, meta, (slot_a, slot_b, offs, sizes, beta)


def kernel(h, x, k, num_batch, batch_seg, W_down, W_up, W_pre, W_e0, W_res,
           _want_trace=False):
    h = np.asarray(h, np.float32)
    x = np.asarray(x, np.float32)
    kk = np.asarray(k, np.float32)
    W_down = np.asarray(W_down, np.float32)
    W_up = np.asarray(W_up, np.float32)
    W_pre = np.asarray(W_pre, np.float32)
    W_e0 = np.asarray(W_e0, np.float32)
    W_res = np.asarray(W_res, np.float32)

    in_maps, meta, (slot_a, slot_b, offs, sizes, beta) = _preprocess(
        h, x, batch_seg, kk, W_down, W_up, W_pre, W_e0, W_res)
    NP = slot_a + slot_b
    T = NP // P

    nc = _get_program(slot_a, slot_b)
    res = bass_utils.run_bass_kernel_spmd(
        nc, in_maps, core_ids=list(range(NCORES)), trace=_want_trace)

    hu = np.zeros((N, E), np.float32)
    dot = np.zeros((N, K), np.float32)
    out_scale = np.float32(1.0 / beta[H])
    for cix in range(NCORES):
        gA, gB, szA, szB = meta[cix]
        r = res.results[cix]
        # huT image [P, 2, NP] -> [2, P, NP] -> [256, NP] -> [NP, 256]
        huT = np.asarray(r["huT"]).reshape(P, 2, NP).transpose(1, 0, 2)
        hu_pad = huT.reshape(2 * P, NP).T  # [NP, E]
        # dot image [P, T, K] -> [T, P, K] -> [NP, K]
        dpad = np.asarray(r["dot_o"]).reshape(P, T, K).transpose(
            1, 0, 2).reshape(NP, K)
        hu[offs[gA]:offs[gA] + szA] = hu_pad[0:szA]
        hu[offs[gB]:offs[gB] + szB] = hu_pad[slot_a:slot_a + szB]
        dot[offs[gA]:offs[gA] + szA] = dpad[0:szA]
        dot[offs[gB]:offs[gB] + szB] = dpad[slot_a:slot_a + szB]
    hu *= out_scale

    if _want_trace:
        kernel._last_results = res  # stash for profiling harnesses
    return (hu, dot, np.float32(1.0))


# revision 36
# speedup vs baseline: 1.0217x; 1.0194x over previous
"""Trainium2 Bass kernel for nn_EwaldBlock (gnn_message_passing).

Sharding: data-parallel over graphs. 16 graphs -> 8 cores, 2 whole graphs per
core (paired large+small, padded to static 128-aligned slots), so the
segment_sum stays core-local. Small weights are replicated.

Per-core device program (feature-major activations [E=2x128 part-tiles, n]):
  dot (PE, fp32) -> sin/cos via range-reduced ACT Sin (fused C|S layout)
  z1 = h @ Wp0.T (f32r), silu; z2 atom-major via activation-stationary
  matmuls, silu; hres = 0.6*h + silu(z2)   (GemNet 1/0.6 & 1/sqrt2 scales
  folded into host-preprocessed weights)
  sf (segment outer-product sums): one K=128 matmul per 128-atom tile emits
  sf_r (psum rows 0:64) and sf_i (rows 64:128) at once
  gather-back: one K=128 matmul per chunk = filt_r.T@cosT + filt_i.T@sinT
  e0 dense + 3 residual layers, feature-major f32r matmuls, pipelined as two
  independent n-streams so ACT(silu) and PE overlap across layers.
"""

import os
import sys
import math
import numpy as np

sys.path.insert(0, "/opt/trn_rl_repo")

import concourse.bass as bass  # noqa: E402,F401
import concourse.mybir as mybir  # noqa: E402
import concourse.tile as tile  # noqa: E402
from concourse import bacc  # noqa: E402
from concourse import bass_utils  # noqa: E402

N, E, B, K, D, H = 8192, 256, 16, 64, 8, 3
NCORES = 8
P = 128
INV_SQRT_2 = 0.7071067811865476
SILU_SCALE = 1.0 / 0.6

F32 = mybir.dt.float32
F32R = mybir.dt.float32r
AF = mybir.ActivationFunctionType
ALU = mybir.AluOpType

TWOPI = float(2 * np.pi)
INV2PI = float(1.0 / (2 * np.pi))
MAGIC = float(3 << 22)  # fp32 round-to-nearest-int magic constant
HALFPI = float(np.pi / 2)


def _nsl(total, base=0):
    """<=512-wide slices, all >=256 (f32r matmuls run 1 cyc/row only at
    free-dim >=256). total must be a multiple of 128 and >=256."""
    out = []
    rem, o = total, 0
    while rem > 0:
        if rem >= 768 or rem <= 512:
            w = min(512, rem)
        else:  # 513..767 -> two 128-aligned pieces, both >=256
            w = (rem // 2 + 127) // 128 * 128
        out.append((base + o, w))
        o += w
        rem -= w
    return out


def build_program(slot_a, slot_b, stage=99, repeat=1):
    """Build the SPMD per-core Bass program. Static in (slot_a, slot_b)."""
    NP = slot_a + slot_b
    T = NP // P
    TA = slot_a // P

    nc = bacc.Bacc("TRN2", target_bir_lowering=False, debug=False,
                   num_devices=NCORES)

    # ---- DRAM I/O (all SBUF partition-major images; contiguous DMAs) ----
    d_hT = nc.dram_tensor("hT", [P, 2 * NP], F32R, kind="ExternalInput")
    d_h06 = nc.dram_tensor("h06", [P, T * E], F32, kind="ExternalInput")
    d_xT = nc.dram_tensor("xT", [3, NP], F32, kind="ExternalInput")
    d_kgT = nc.dram_tensor("kgT", [3, 2 * K], F32, kind="ExternalInput")
    d_kf2 = nc.dram_tensor("kf2", [P, E], F32R, kind="ExternalInput")
    d_idn = nc.dram_tensor("idn", [P, P], F32R, kind="ExternalInput")
    d_wp0 = nc.dram_tensor("wp0", [P, 4 * P], F32R, kind="ExternalInput")
    d_wp1 = nc.dram_tensor("wp1", [P, 2 * E], F32R, kind="ExternalInput")
    d_we0 = nc.dram_tensor("we0", [P, 4 * P], F32R, kind="ExternalInput")
    d_wres = nc.dram_tensor("wres", [P, 8 * H * P], F32R,
                            kind="ExternalInput")
    d_huT = nc.dram_tensor("huT", [P, 2 * NP], F32R, kind="ExternalOutput")
    d_dot = nc.dram_tensor("dot_o", [P, T * K], F32, kind="ExternalOutput")

    # two independent n-streams through the feature-major MLP chain.
    # Matmul psum writes must stay inside one 512-f32 bank, so slices are
    # plain 512-chunks relative to the half's psum tile.
    halves = [(0, 512), (512, NP - 512)]
    half_tag = ["mlpA", "mlpB"]

    def _bank_slices(ho, hw):
        out, o = [], 0
        while o < hw:
            w = min(512, hw - o)
            out.append((ho + o, w))
            o += w
        return out

    half_slices = [_bank_slices(ho, hw) for (ho, hw) in halves]
    # gather chunks each get their own psum tile (offset 0): keep all >=256
    g_chunks = [_nsl(slot_a, 0), _nsl(slot_b, slot_a)]

    with tile.TileContext(nc) as tc:
      with tc.tile_pool(name="sb", bufs=1) as sb, \
           tc.tile_pool(name="ps", bufs=1, space="PSUM") as ps:
        for _rep in range(repeat):
            # ---- inputs to SBUF, ordered by first use ----
            halfpi = sb.tile([P, 1], F32, tag="halfpi", name="halfpi")
            nc.vector.memset(halfpi[:], HALFPI)
            xT = sb.tile([3, NP], F32, tag="xT", name="xT")
            nc.sync.dma_start(xT[:], d_xT.ap())
            kg = sb.tile([3, 2, K], F32, tag="kg", name="kg")
            nc.sync.dma_start(kg.rearrange("p g k -> p (g k)")[:],
                              d_kgT.ap())
            wp0 = sb.tile([P, 4, P], F32R, tag="wp0", name="wp0")
            nc.sync.dma_start(wp0.rearrange("p q f -> p (q f)")[:],
                              d_wp0.ap())
            # hT quartered: [ei0-A, ei1-A, ei0-B, ei1-B] so z1's A-half
            # matmuls can start after ~2 quarters
            hT = sb.tile([P, 2, NP], F32R, tag="hT", name="hT")
            for (qo, qw) in ((0, 512), (512, NP - 512)):
                for ei in range(2):
                    nc.sync.dma_start(
                        hT[:, ei, qo:qo + qw],
                        d_hT.ap()[:, ei * NP + qo:ei * NP + qo + qw])
            idn = sb.tile([P, P], F32R, tag="idn", name="idn")
            nc.sync.dma_start(idn[:], d_idn.ap())
            wp1 = sb.tile([P, 2, E], F32R, tag="wp1", name="wp1")
            nc.sync.dma_start(wp1.rearrange("p q f -> p (q f)")[:],
                              d_wp1.ap())
            h06 = sb.tile([P, T, E], F32, tag="h06", name="h06")
            nc.sync.dma_start(h06.rearrange("p t e -> p (t e)")[:],
                              d_h06.ap())
            kf2 = sb.tile([P, E], F32R, tag="kf2", name="kf2")
            nc.sync.dma_start(kf2[:], d_kf2.ap())
            we0 = sb.tile([P, 4, P], F32R, tag="we0", name="we0")
            nc.sync.dma_start(we0.rearrange("p q f -> p (q f)")[:],
                              d_we0.ap())
            wres = sb.tile([P, 8 * H, P], F32R, tag="wres", name="wres")
            nc.sync.dma_start(wres.rearrange("p q f -> p (q f)")[:],
                              d_wres.ap())

            # ---- stage D: dot products + trig (atom-major) ----
            # CS_at[:, t, 0:64] = cos, [:, t, 64:128] = sin  (fused so the
            # sf matmul emits sf_r on psum rows 0:64 and sf_i on 64:128)
            CS_at = sb.tile([P, T, 2 * K], F32R, tag="CS_at", name="CS_at")
            dot_sb = sb.tile([P, T, K], F32, tag="dot_sb", name="dot_sb")
            tiles = list(range(T))
            sin_insts = []
            for p0 in range(0, T, 4):
                pk = tiles[p0:p0 + 4]
                w = len(pk) * K
                dps = ps.tile([P, 512], F32,
                              tag=("small" if (p0 // 4) % 2 == 0 else "sf"),
                              name=f"dot_{p0}")
                for j, t in enumerate(pk):
                    g = 0 if t < TA else 1
                    nc.tensor.matmul(
                        dps[:, j * K:(j + 1) * K],
                        lhsT=xT[0:3, t * P:(t + 1) * P],
                        rhs=kg[0:3, g, :],
                        start=True, stop=True)
                fl = slice(p0 * K, p0 * K + w)  # flattened (T,K) slice
                dflat = dot_sb.rearrange("p t k -> p (t k)")
                t1 = sb.tile([P, 512], F32, tag="t1", bufs=2,
                             name=f"t1_{p0}")
                nc.vector.tensor_scalar(t1[:, :w], dps[:, :w], INV2PI,
                                        MAGIC, ALU.mult, ALU.add)
                k1 = sb.tile([P, 512], F32, tag="k1", bufs=2,
                             name=f"k1_{p0}")
                nc.vector.tensor_scalar(k1[:, :w], t1[:, :w], MAGIC, None,
                                        ALU.subtract)
                rr = sb.tile([P, 512], F32, tag="rr", bufs=2,
                             name=f"rr_{p0}")
                nc.vector.scalar_tensor_tensor(rr[:, :w], k1[:, :w],
                                               -TWOPI, dps[:, :w],
                                               ALU.mult, ALU.add)
                kc = sb.tile([P, 512], F32, tag="kc", bufs=2,
                             name=f"kc_{p0}")
                nc.vector.tensor_scalar(kc[:, :w], rr[:, :w], HALFPI, None,
                                        ALU.is_gt)
                rc = sb.tile([P, 512], F32, tag="rc", bufs=2,
                             name=f"rc_{p0}")
                nc.vector.scalar_tensor_tensor(rc[:, :w], kc[:, :w],
                                               -TWOPI, rr[:, :w],
                                               ALU.mult, ALU.add)
                sin_insts.append(nc.scalar.activation(
                    CS_at[:, p0:p0 + len(pk), K:2 * K],
                    rr[:, :w].rearrange("p (t k) -> p t k", k=K), AF.Sin))
                sin_insts.append(nc.scalar.activation(
                    CS_at[:, p0:p0 + len(pk), 0:K],
                    rc[:, :w].rearrange("p (t k) -> p t k", k=K),
                    AF.Sin, bias=halfpi[:]))
                nc.vector.tensor_copy(dflat[:, fl], dps[:, :w])
            nc.sync.dma_start(d_dot.ap(),
                              dot_sb.rearrange("p t k -> p (t k)")[:])

            # ---- stage Z1: z1 = hT @ Wp0 (feature-major), y1 = silu ----
            y1T = sb.tile([P, 2, NP], F32R, tag="y1T", name="y1T")
            for hf, (ho, hw) in enumerate(halves):
                for eo in range(2):
                    z1p = ps.tile([P, hw], F32, tag=half_tag[hf], bufs=2,
                                  name=f"z1_{hf}{eo}")
                    for ei in range(2):
                        for (o, w) in half_slices[hf]:
                            nc.tensor.matmul(z1p[:, o - ho:o - ho + w],
                                             lhsT=wp0[:, ei * 2 + eo, :],
                                             rhs=hT[:, ei, o:o + w],
                                             start=(ei == 0),
                                             stop=(ei == 1))
                    _silu1 = nc.scalar.activation(y1T[:, eo, ho:ho + hw],
                                                  z1p[:], AF.Silu)
                    # keep all Sin ACTs before any Silu: one table switch
                    tile.add_dep_helper(_silu1.ins, sin_insts[-1].ins,
                                        sync=False,
                                        reason="act table grouping")

            # ---- stage T: CST = transpose(CS_at) ----
            # CST rows 0:64 = cos^T, rows 64:128 = sin^T (k-major)
            CST = sb.tile([P, NP], F32R, tag="CST", name="CST")
            for p0 in range(0, T, 4):
                pk = tiles[p0:p0 + 4]
                w = len(pk) * P
                tp = ps.tile([P, 512], F32R, tag="small", name=f"tp_{p0}")
                for j, t in enumerate(pk):
                    nc.tensor.transpose(tp[:, j * P:(j + 1) * P],
                                        CS_at[:, t, :], idn[:])
                nc.vector.tensor_copy(CST[:, p0 * P:p0 * P + w], tp[:, :w])

            # ---- stage Z2 + hres + sf ----
            hres = sb.tile([P, T, E], F32R, tag="hres", name="hres")
            sfp = ps.tile([P, 512], F32, tag="sf", name="sfp")
            h06f = h06.rearrange("p t e -> p (t e)")
            hresf = hres.rearrange("p t e -> p (t e)")
            hu0T = sb.tile([P, 2, NP], F32R, tag="hu0T", name="hu0T")
            filt = [None, None]

            def emit_filt_gather(g):
                fg = sb.tile([P, E], F32R, tag=f"filt{g}", name=f"filt_{g}")
                nc.vector.tensor_mul(fg[:], sfp[:, g * E:(g + 1) * E],
                                     kf2[:])
                filt[g] = fg
                for eo in range(2):
                    for gi, (o, w) in enumerate(g_chunks[g]):
                        _tg = "mlpA" if (eo + gi) % 2 == 0 else "small"
                        gp = ps.tile([P, 512], F32, tag=_tg,
                                     bufs=(1 if _tg == "small" else 2),
                                     name=f"g_{g}{eo}{o}")
                        nc.tensor.matmul(gp[:, :w],
                                         lhsT=fg[:, eo * P:(eo + 1) * P],
                                         rhs=CST[:, o:o + w],
                                         start=True, stop=True)
                        nc.vector.tensor_copy(hu0T[:, eo, o:o + w],
                                              gp[:, :w])

            z2packs = [tiles[p0:p0 + 4] for p0 in range(0, T, 4)]
            sf_last = [None]
            for pk in z2packs:
                p0 = pk[0]
                w = len(pk) * E
                zp = ps.tile([P, 1024], F32, tag="mlpB", bufs=2,
                             name=f"z2_{p0}")
                for j, t in enumerate(pk):
                    for ei in range(2):
                        nc.tensor.matmul(
                            zp[:, j * E:(j + 1) * E],
                            lhsT=y1T[:, ei, t * P:(t + 1) * P],
                            rhs=wp1[:, ei, :],
                            start=(ei == 0), stop=(ei == 1))
                t2 = sb.tile([P, 1024], F32, tag="t2", bufs=2,
                             name=f"t2_{p0}")
                nc.scalar.activation(t2[:, :w], zp[:, :w], AF.Silu)
                fl = slice(p0 * E, p0 * E + w)
                nc.vector.tensor_add(hresf[:, fl], t2[:, :w], h06f[:, fl])
                for t in pk:
                    g = 0 if t < TA else 1
                    first = t in (0, TA)
                    last = t in (TA - 1, T - 1)
                    # sf[:, g*256:]: rows 0:64 = sf_r, rows 64:128 = sf_i
                    _sfmm = nc.tensor.matmul(sfp[:, g * E:(g + 1) * E],
                                             lhsT=CS_at[:, t, :],
                                             rhs=hres[:, t, :],
                                             start=first, stop=last)
                    if first and g == 1 and sf_last[0] is not None:
                        # B's accumulation group must open after A's closes
                        # (one psum bank holds both zero regions)
                        tile.add_dep_helper(_sfmm.ins, sf_last[0].ins,
                                            sync=False,
                                            reason="sf bank group order")
                    if last:
                        sf_last[0] = _sfmm
                        emit_filt_gather(g)

            # ---- e0 dense + 3 residual layers (feature-major, 2 streams) --
            def mm_set(lhs_w, qbase, rhs, hf, eo, name):
                ho, hw = halves[hf]
                pp = ps.tile([P, hw], F32, tag=half_tag[hf], bufs=2,
                             name=f"{name}_{hf}{eo}")
                for ei in range(2):
                    for (o, w) in half_slices[hf]:
                        nc.tensor.matmul(pp[:, o - ho:o - ho + w],
                                         lhsT=lhs_w[:, qbase + ei * 2 + eo,
                                                    :],
                                         rhs=rhs[:, ei, o:o + w],
                                         start=(ei == 0), stop=(ei == 1))
                return pp

            x0 = sb.tile([P, 2, NP], F32R, tag="xhat", bufs=2, name="x0")
            for hf, (ho, hw) in enumerate(halves):
                for eo in range(2):
                    pp = mm_set(we0, 0, hu0T, hf, eo, "e0")
                    nc.scalar.activation(x0[:, eo, ho:ho + hw], pp[:],
                                         AF.Silu)

            xcur = x0
            for i in range(H):
                gamma = float(math.sqrt(2.0) ** i)
                ta = sb.tile([P, 2, NP], F32R, tag="tA", bufs=2,
                             name=f"ta_{i}")
                for hf, (ho, hw) in enumerate(halves):
                    for eo in range(2):
                        pp = mm_set(wres, 8 * i, xcur, hf, eo, f"ra{i}")
                        nc.scalar.activation(ta[:, eo, ho:ho + hw], pp[:],
                                             AF.Silu)
                xn = sb.tile([P, 2, NP], F32R, tag="xhat", bufs=2,
                             name=f"x{i + 1}")
                last = i == H - 1
                for hf, (ho, hw) in enumerate(halves):
                    for eo in range(2):
                        pp = mm_set(wres, 8 * i + 4, ta, hf, eo, f"rb{i}")
                        tb = sb.tile([P, 640], F32, tag="tB", bufs=3,
                                     name=f"tb_{i}{hf}{eo}")
                        nc.scalar.activation(tb[:, :hw], pp[:], AF.Silu)
                        nc.vector.scalar_tensor_tensor(
                            xn[:, eo, ho:ho + hw], tb[:, :hw], gamma,
                            xcur[:, eo, ho:ho + hw], ALU.mult, ALU.add)
                        if last:
                            nc.sync.dma_start(
                                d_huT.ap()[:, eo * NP + ho:
                                           eo * NP + ho + hw],
                                xn[:, eo, ho:ho + hw])
                xcur = xn

    nc.compile()
    return nc


_PROG_CACHE = {}


def _get_program(slot_a, slot_b):
    key = (slot_a, slot_b)
    if key not in _PROG_CACHE:
        _PROG_CACHE[key] = build_program(slot_a, slot_b)
    return _PROG_CACHE[key]


def _preprocess(h, x, batch_seg, kk, W_down, W_up, W_pre, W_e0, W_res):
    """Host-side sharding + weight folding. Returns (in_maps, meta)."""
    c = SILU_SCALE
    s = INV_SQRT_2

    bs = np.asarray(batch_seg).astype(np.int64)
    offs = np.searchsorted(bs, np.arange(B + 1))
    sizes = np.diff(offs)

    # pair graphs: i-th largest with i-th smallest
    order = np.argsort(-sizes, kind="stable")
    pairs = [(int(order[i]), int(order[2 * NCORES - 1 - i]))
             for i in range(NCORES)]
    big = [p[0] for p in pairs]
    small = [p[1] for p in pairs]
    slot_a = int(math.ceil(max(sizes[g] for g in big) / P) * P)
    slot_b = int(math.ceil(max(max(sizes[g] for g in small), 1) / P) * P)
    slot_a = max(slot_a, 256)
    slot_b = max(slot_b, 256)
    NP = slot_a + slot_b
    T = NP // P

    # ---- shared weights (folded scales) ----
    beta = [0.6 * (math.sqrt(2.0) ** i) for i in range(H + 1)]
    wp0 = np.ascontiguousarray(W_pre[0].T)                   # [ei, eo]
    wp1 = np.ascontiguousarray((c * W_pre[1]).T)
    we0 = np.ascontiguousarray(W_e0.T)
    wres_l = []
    for i in range(H):
        wres_l.append(np.ascontiguousarray((W_res[i, 0] / beta[i]).T))
        wres_l.append(np.ascontiguousarray((c * W_res[i, 1]).T))

    def blocks4(w):  # [256,256] -> [4,128,128] (q = ei*2+eo)
        return np.ascontiguousarray(
            w.reshape(2, P, 2, P).transpose(0, 2, 1, 3).reshape(4, P, P))

    wp0_b = blocks4(wp0)
    we0_b = blocks4(we0)
    wres_b = np.empty((8 * H, P, P), np.float32)
    for i in range(H):
        wres_b[8 * i:8 * i + 4] = blocks4(wres_l[2 * i])
        wres_b[8 * i + 4:8 * i + 8] = blocks4(wres_l[2 * i + 1])
    wp1_b = np.ascontiguousarray(wp1.reshape(2, P, E))

    def pmajor(a):  # [Q, P, F] -> SBUF image [P, Q*F]
        q, p, f = a.shape
        return np.ascontiguousarray(
            a.transpose(1, 0, 2).reshape(p, q * f)).astype(np.float32)

    kfilter = (W_up @ W_down).T                              # [K, E]
    kf_dev = (0.01 * c * s) * kfilter
    kf2 = np.ascontiguousarray(np.concatenate([kf_dev, kf_dev], axis=0)
                               ).astype(np.float32)          # [128, E]
    idn = np.eye(P, dtype=np.float32)

    shared = {
        "kf2": kf2, "idn": idn, "wp0": pmajor(wp0_b),
        "wp1": pmajor(wp1_b), "we0": pmajor(we0_b),
        "wres": pmajor(wres_b),
    }

    in_maps = []
    meta = []
    for cix in range(NCORES):
        gA, gB = pairs[cix]
        szA, szB = int(sizes[gA]), int(sizes[gB])
        hpad = np.zeros((NP, E), np.float32)
        xpad = np.zeros((NP, 3), np.float32)
        hpad[0:szA] = h[offs[gA]:offs[gA] + szA]
        xpad[0:szA] = x[offs[gA]:offs[gA] + szA]
        hpad[slot_a:slot_a + szB] = h[offs[gB]:offs[gB] + szB]
        xpad[slot_a:slot_a + szB] = x[offs[gB]:offs[gB] + szB]
        m = dict(shared)
        m["hT"] = pmajor(hpad.T.reshape(2, P, NP))
        m["h06"] = pmajor((hpad / c).reshape(T, P, E))
        m["xT"] = np.ascontiguousarray(xpad.T)
        m["kgT"] = np.ascontiguousarray(
            np.stack([kk[gA].T, kk[gB].T], axis=1).reshape(3, 2 * K)
        ).astype(np.float32)
        in_maps.append(m)
        meta.append((gA, gB, szA, szB))

    return in_maps, meta, (slot_a, slot_b, offs, sizes, beta)


def kernel(h, x, k, num_batch, batch_seg, W_down, W_up, W_pre, W_e0, W_res,
           _want_trace=False):
    h = np.asarray(h, np.float32)
    x = np.asarray(x, np.float32)
    kk = np.asarray(k, np.float32)
    W_down = np.asarray(W_down, np.float32)
    W_up = np.asarray(W_up, np.float32)
    W_pre = np.asarray(W_pre, np.float32)
    W_e0 = np.asarray(W_e0, np.float32)
    W_res = np.asarray(W_res, np.float32)

    in_maps, meta, (slot_a, slot_b, offs, sizes, beta) = _preprocess(
        h, x, batch_seg, kk, W_down, W_up, W_pre, W_e0, W_res)
    NP = slot_a + slot_b
    T = NP // P

    nc = _get_program(slot_a, slot_b)
    res = bass_utils.run_bass_kernel_spmd(
        nc, in_maps, core_ids=list(range(NCORES)), trace=_want_trace)

    hu = np.zeros((N, E), np.float32)
    dot = np.zeros((N, K), np.float32)
    out_scale = np.float32(1.0 / beta[H])
    for cix in range(NCORES):
        gA, gB, szA, szB = meta[cix]
        r = res.results[cix]
        # huT image [P, 2, NP] -> [2, P, NP] -> [256, NP] -> [NP, 256]
        huT = np.asarray(r["huT"]).reshape(P, 2, NP).transpose(1, 0, 2)
        hu_pad = huT.reshape(2 * P, NP).T  # [NP, E]
        # dot image [P, T, K] -> [T, P, K] -> [NP, K]
        dpad = np.asarray(r["dot_o"]).reshape(P, T, K).transpose(
            1, 0, 2).reshape(NP, K)
        hu[offs[gA]:offs[gA] + szA] = hu_pad[0:szA]
        hu[offs[gB]:offs[gB] + szB] = hu_pad[slot_a:slot_a + szB]
        dot[offs[gA]:offs[gA] + szA] = dpad[0:szA]
        dot[offs[gB]:offs[gB] + szB] = dpad[slot_a:slot_a + szB]
    hu *= out_scale

    if _want_trace:
        kernel._last_results = res  # stash for profiling harnesses
    return (hu, dot, np.float32(1.0))
